# revision 15
# baseline (speedup 1.0000x reference)
"""BayesianLinear (y = x @ (mu + softplus(rho) * eps).T + bias) on 8 TRN2 cores.

Column-parallel sharding: each core owns OUT_F/8 = 512 output features.

Host-side prep is pure layout/precision staging (no reference math):
  - x is cast to bf16 and pre-tiled into the SBUF layout the TensorEngine
    needs for its stationary operand: x_t[bt, pi, po, bi] = x[bt*128+bi,
    po*128+pi], so each 128-row batch tile is one contiguous 1 MiB DMA.
  - weight params are transposed to [in_f, o_sh], tiled per 128-row
    K-block, and PACKED into one uint8 tensor [128, KB, 2048]:
    eps bf16 (1024 B) | mu int8 (512 B) | rho uint8 (512 B) per block.
    mu/rho ship as affine-quantized 8-bit codes (scales in a tiny qp
    tensor): uniform 8-bit beats fp8 ~3x in rms error for Gaussian data
    and halves the packed-weight HBM traffic (12.6 -> 8.4 MiB/core) --
    the construction phase is HBM-bound, not compute-bound.

Device per core:
  1. Construction units (2 single K-blocks first for a short critical
     path, then 15 pairs): one packed DMA (GPSIMD SWDGE queue), then
     softplus(rho) = Ln(1 + Exp(rho)) on ACT -- the uint8 rho dequant
     rides Exp's free affine (out = f(scale*in + bias), scale/bias as
     per-partition APs from qp). DVE: mul by eps (bf16 2x mode), then
     one fused scalar_tensor_tensor (mu_i8 * mu_scale) + sp_eps writing
     bf16 straight into the resident W^T tile [128, 32, 512].
  2. bias row = bias_mu + softplus(bias_rho) * bias_eps (fp32, tiny),
     broadcast across partitions with one K=1 matmul against a ones row
     mid-wave so the in-order PE stream never head-of-line blocks on it.
  3. First 7 batch tiles run as a k-WAVEFRONT across the 7 PSUM banks:
     wave w issues tile i's matmul for k = w - i. Tile 0's first matmul
     only needs W^T block 0 + one 256 KiB x chunk, so real work starts
     ~12 us in (vs ~27 us for the k-major group), and the PE consumes
     W^T blocks no faster than construction produces them. x chunks are
     DMA'd in need-order with a 10-wave lookahead so the packed-weight
     DMAs get their required HBM share.
  4. A short PE warmup (dummy K=1 matmuls, no DMA deps) bridges the
     framework preamble to first-data-ready so the HAM clock gate is
     already at 8/8 when the real stream starts.
  5. Remaining 56 tiles stream one PSUM bank each: one 1 MiB x DMA, 32
     accumulating bf16 matmuls into PSUM [128, 512] fp32, DVE eviction
     fused with the bias add, DMA out.
"""

import numpy as np
import ml_dtypes

import concourse.bacc as bacc
import concourse.mybir as mybir
import concourse.tile as tile
from concourse.bass_utils import run_bass_kernel_spmd

BATCH = 8192
IN_F = 4096
OUT_F = 4096
N_CORES = 8
P = 128

_NC_CACHE = {}

PKB = 2048  # packed bytes per partition per K-block: eps 1024 | mu 512 | rho 512
WARM = 14  # PE warmup matmuls
LOOKAHEAD = 5  # waves of x-chunk DMA lookahead
CH = 4  # x chunks per strip (8 K-blocks each)


def build_nc(batch=BATCH, in_f=IN_F, o_sh=OUT_F // N_CORES, qparams=(1.0, 0.0, 1.0)):
    """qparams = (rho_scale, rho_min, mu_scale), shared by all cores
    (global quantization grid), baked as compile-time immediates so the
    first Exp doesn't wait on a parameter DMA."""
    rho_sc, rho_min, mu_sc = (float(v) for v in qparams)
    KB = in_f // P  # K-blocks of 128 along the contraction dim
    BT = batch // P  # 128-row output tiles

    nc = bacc.Bacc(
        "TRN2",
        target_bir_lowering=False,
        debug=False,
        enable_asserts=False,
        num_devices=N_CORES,
    )
    bf16 = mybir.dt.bfloat16
    f32 = mybir.dt.float32
    u8 = mybir.dt.uint8
    i8 = mybir.dt.int8

    x = nc.declare_dram_parameter("x_t", [BT, P, KB, P], bf16, isOutput=False)
    wpk = nc.declare_dram_parameter("wpk_t", [P, KB, PKB], u8, isOutput=False)
    bpk = nc.declare_dram_parameter("bpk", [1, 3 * o_sh], f32, isOutput=False)
    y = nc.declare_dram_parameter("y", [batch, o_sh], f32, isOutput=True)

    act_exp = mybir.ActivationFunctionType.Exp
    act_ln = mybir.ActivationFunctionType.Ln
    op_mult = mybir.AluOpType.mult
    op_add = mybir.AluOpType.add

    # construction units: small first (short first-ready chain, and the
    # early wavefront demands one fresh block per ~1.1 us), quads later —
    # bigger ACT passes amortize the ~352-cycle fixed cost; the serial
    # ACT chain is the construction floor.
    units = [(0, 1), (1, 1), (2, 2), (4, 2), (6, 2)]
    b = 8
    while b < KB:
        s = min(4, KB - b)
        units.append((b, s))
        b += s

    with tile.TileContext(nc) as tc:
        with (
            tc.tile_pool(name="const", bufs=1) as const,
            tc.tile_pool(name="wcons", bufs=6) as wcons,
            # bufs=8: 7 live group strips + ONE streaming-prefetch slot.
            # More slots would let 1 MiB streaming prefetches flood the
            # DMA rings mid-group and starve the packed-weight DMAs the
            # ACT pipeline is waiting on (measured: wpk latency 11 us,
            # ACT blocked 27 us). Slots open progressively as group
            # strips retire, so streaming runway builds during the
            # group tail exactly when HBM frees up.
            tc.tile_pool(name="xin", bufs=8) as xin,
            tc.tile_pool(name="yout", bufs=4) as yout,
            tc.tile_pool(name="psum", bufs=7, space="PSUM") as psum_pool,
            tc.tile_pool(name="bpsum", bufs=1, space="PSUM") as bias_psum,
        ):
            bias_sb = const.tile([P, o_sh], f32, tag="bias_sb")
            bias_bf = const.tile([1, o_sh], bf16, tag="bias_bf")
            ones = const.tile([1, P], bf16, tag="ones")
            nc.vector.memset(ones[:], 1.0)
            wones = const.tile([1, o_sh], bf16, tag="wones")
            nc.vector.memset(wones[:], 1.0)
            # rho_min as a memset const (activation bias must be an AP;
            # a memset has no DMA dependency, unlike a parameter load)
            rho_min_sb = const.tile([P, 1], f32, tag="rho_min_sb")
            nc.vector.memset(rho_min_sb[:], rho_min)
            # dummy Exp so walrus's ACT_TABLE_LOAD runs during the idle
            # preamble instead of serializing behind the first wpk DMA
            act_scratch = const.tile([P, 1], f32, tag="act_scratch")
            nc.scalar.activation(act_scratch[:], rho_min_sb[:], act_exp)

            # PE warmup: dummy matmuls with no DMA deps bridge the
            # ~6.5us framework preamble to first-data-ready (~12us).
            # K must be 128: the HAM clock gate watches PE-array
            # activity, and a K=1 matmul (1 of 128 rows) stays below its
            # busy threshold — measured traces showed K=1 warmups never
            # flipped the gate and the first ~20 real matmuls ran at the
            # cold 1.2 GHz clock.
            warm_w = const.tile([P, P], bf16, tag="warm_w")
            nc.vector.memset(warm_w[:], 0.001)
            warm_r = const.tile([P, o_sh], bf16, tag="warm_r")
            nc.vector.memset(warm_r[:], 0.001)
            warm_ps = bias_psum.tile([P, o_sh], f32, tag="bias_ps", name="warm_ps")
            for w in range(WARM):
                nc.tensor.matmul(warm_ps[:], lhsT=warm_w[:], rhs=warm_r[:])

            def emit_bias_row():
                # one packed DMA (mu | rho | eps) on the gpsimd ring so
                # the sync ring's head stays dedicated to x chunks
                b_pk = const.tile([1, 3 * o_sh], f32, tag="b_pk")
                nc.gpsimd.dma_start(out=b_pk[:], in_=bpk[:])
                b_mu = b_pk[:, 0:o_sh]
                b_rho = b_pk[:, o_sh : 2 * o_sh]
                b_eps = b_pk[:, 2 * o_sh : 3 * o_sh]
                b_sp = const.tile([1, o_sh], f32, tag="b_sp")
                nc.scalar.activation(b_sp[:], b_rho[:], act_exp)
                nc.scalar.activation(b_sp[:], b_sp[:], act_ln, bias=1.0)
                nc.vector.tensor_mul(out=b_sp[:], in0=b_sp[:], in1=b_eps[:])
                nc.vector.tensor_add(out=bias_bf[:], in0=b_sp[:], in1=b_mu[:])

            # ---- W^T constructed in place. Per unit: one packed DMA,
            # Exp with fused uint8-rho dequant, Ln, eps-mul (bf16 2x),
            # fused (mu_i8 * scale) + sp_eps -> bf16 W^T block.
            WT = const.tile([P, KB, o_sh], bf16, tag="WT")
            for ui, (ub, us) in enumerate(units):
                pk = wcons.tile([P, us, PKB], u8, tag="pk")
                # unit 0 rides the sync ring's head (HWDGE, in front of
                # the x chunks) — it gates the whole construction chain
                dma_eng = nc.sync if ui == 0 else nc.gpsimd
                dma_eng.dma_start(out=pk[:], in_=wpk[:, ub : ub + us, :])
                eps_v = pk[:, :, 0 : 2 * o_sh].bitcast(bf16)
                mu_v = pk[:, :, 2 * o_sh : 3 * o_sh].bitcast(i8)
                rho_v = pk[:, :, 3 * o_sh : 4 * o_sh]
                sp = wcons.tile([P, us, o_sh], bf16, tag="sp")
                nc.scalar.activation(
                    sp[:], rho_v[:], act_exp, bias=rho_min_sb[:], scale=rho_sc
                )
                nc.scalar.activation(sp[:], sp[:], act_ln, bias=1.0)
                nc.vector.tensor_mul(out=sp[:], in0=sp[:], in1=eps_v[:])
                nc.vector.scalar_tensor_tensor(
                    out=WT[:, ub : ub + us, :],
                    in0=mu_v[:],
                    scalar=mu_sc,
                    in1=sp[:],
                    op0=op_mult,
                    op1=op_add,
                )
                if ui == 2:
                    emit_bias_row()

            def body_tail(ps, bt):
                y_sb = yout.tile([P, o_sh], f32, tag="y_sb")
                nc.vector.tensor_add(out=y_sb[:], in0=ps[:], in1=bias_sb[:])
                nc.sync.dma_start(out=y[bt * P : (bt + 1) * P, :], in_=y_sb[:])

            # ---- first GROUP tiles run as a k-wavefront across PSUM
            # banks: wave w = tile i's matmul for k = w - i. Tile 0's
            # k=0 matmul needs only W^T block 0 + one x chunk.
            GROUP = min(7, BT)
            KC = KB // CH  # K-blocks per x chunk
            xts = []
            pss = []
            for bt in range(GROUP):
                xT = xin.tile([P, KB, P], bf16, tag="xT", name=f"xT_g{bt}")
                xts.append(xT)
                ps = psum_pool.tile([P, o_sh], f32, tag="ps", name=f"ps_g{bt}")
                pss.append(ps)

            # x chunk (i, c) is first read at wave i + c*KC; DMA in need
            # order with LOOKAHEAD waves of headroom so the packed-weight
            # DMAs keep their HBM share.
            chunks = sorted(
                ((i + c * KC, i, c) for i in range(GROUP) for c in range(CH))
            )

            def issue_chunks_through(wave):
                while chunks and chunks[0][0] <= wave:
                    _, i, c = chunks.pop(0)
                    ks = slice(c * KC, (c + 1) * KC)
                    nc.sync.dma_start(out=xts[i][:, ks, :], in_=x[i, :, ks, :])

            issue_chunks_through(LOOKAHEAD - 1)
            NWAVE = KB + GROUP - 1
            for w in range(NWAVE):
                issue_chunks_through(w + LOOKAHEAD)
                for i in range(GROUP):
                    k = w - i
                    if 0 <= k < KB:
                        nc.tensor.matmul(
                            pss[i][:],
                            lhsT=xts[i][:, k, :],
                            rhs=WT[:, k, :],
                            start=(k == 0),
                            stop=(k == KB - 1),
                        )
                if w == 12:
                    # bias broadcast: [128, o_sh] = ones.T @ bias_bf.
                    # Mid-stream so the in-order PE queue never blocks
                    # on the bias chain; ready long before 1st eviction.
                    bias_ps = bias_psum.tile(
                        [P, o_sh], f32, tag="bias_ps", name="bias_ps"
                    )
                    nc.tensor.matmul(bias_ps[:], lhsT=ones[:], rhs=bias_bf[:])
                    nc.vector.tensor_copy(out=bias_sb[:], in_=bias_ps[:])
                gi = w - (KB - 1)
                if 0 <= gi < GROUP:
                    body_tail(pss[gi], gi)

            # ---- remaining tiles stream one PSUM bank each
            for bt in range(GROUP, BT):
                xT = xin.tile([P, KB, P], bf16, tag="xT")
                nc.sync.dma_start(out=xT[:], in_=x[bt])
                ps = psum_pool.tile([P, o_sh], f32, tag="ps")
                for k in range(KB):
                    nc.tensor.matmul(
                        ps[:],
                        lhsT=xT[:, k, :],
                        rhs=WT[:, k, :],
                        start=(k == 0),
                        stop=(k == KB - 1),
                    )
                body_tail(ps, bt)

    # Skip bacc's pre-placed InstLoadActFuncSet: on large graphs walrus's
    # parallel-pass fork can separate the hoisted load from its activations
    # ("No Act func set exist for this instruction"); walrus's own lower_act
    # placement handles forked subgraphs correctly.
    nc.insert_act_table_loads = lambda: None
    nc.compile()
    return nc


def _prep_x(x):
    """[batch, in_f] fp32 -> bf16 tiled [BT, 128, KB, 128] with
    x_t[bt, pi, po, bi] = x[bt*128 + bi, po*128 + pi]."""
    batch, in_f = x.shape
    xb = x.astype(ml_dtypes.bfloat16)
    xb = xb.reshape(batch // P, P, in_f // P, P)  # [bt, bi, po, pi]
    return np.ascontiguousarray(xb.transpose(0, 3, 2, 1))  # [bt, pi, po, bi]


def _tile_w(w):
    """[o_sh, in_f] -> tiled [KB, 128, o_sh] with w_t[k, pi, o] = w[o, k*128 + pi]."""
    o_sh, in_f = w.shape
    return np.ascontiguousarray(w.T.reshape(in_f // P, P, o_sh))


def _prep_wpk(wmu, wrho, weps, mu_sc, rho_sc, rho_min):
    """Pack eps (bf16 bytes), mu (int8 codes), rho (uint8 codes) into one
    uint8 [128, KB, 2048] tensor (global quantization grids)."""
    eps_t = _tile_w(weps).astype(ml_dtypes.bfloat16)  # [KB, P, o]
    mu_t = _tile_w(wmu)
    rho_t = _tile_w(wrho)

    mu_c = np.clip(np.round(mu_t / mu_sc), -127, 127).astype(np.int8)
    rho_c = np.clip(np.round((rho_t - rho_min) / rho_sc), 0, 255).astype(np.uint8)

    kb, p, o = mu_t.shape
    pk = np.concatenate(
        [
            eps_t.view(np.uint8).reshape(kb, p, 2 * o),
            mu_c.view(np.uint8),
            rho_c,
        ],
        axis=2,
    )  # [KB, P, 4*o]
    return np.ascontiguousarray(pk.transpose(1, 0, 2))


def quant_params(weight_mu, weight_rho):
    """Global (all-core) quantization grids for mu/rho — shared so they
    can be baked into the SPMD program as immediates."""
    wmu = np.asarray(weight_mu, dtype=np.float32)
    wrho = np.asarray(weight_rho, dtype=np.float32)
    mu_sc = max(float(np.abs(wmu).max()) / 127.0, 1e-30)
    rmin = float(wrho.min())
    rho_sc = max((float(wrho.max()) - rmin) / 255.0, 1e-30)
    return rho_sc, rmin, mu_sc


def make_in_maps(x, weight_mu, weight_rho, bias_mu, bias_rho, weight_eps, bias_eps):
    o_sh = OUT_F // N_CORES
    rho_sc, rho_min, mu_sc = quant_params(weight_mu, weight_rho)
    x_t = _prep_x(np.asarray(x, dtype=np.float32))
    wmu = np.asarray(weight_mu, dtype=np.float32)
    wrho = np.asarray(weight_rho, dtype=np.float32)
    weps = np.asarray(weight_eps, dtype=np.float32)
    bmu = np.asarray(bias_mu, dtype=np.float32).reshape(1, -1)
    brho = np.asarray(bias_rho, dtype=np.float32).reshape(1, -1)
    beps = np.asarray(bias_eps, dtype=np.float32).reshape(1, -1)

    in_maps = []
    for c in range(N_CORES):
        rs = slice(c * o_sh, (c + 1) * o_sh)
        in_maps.append(
            {
                "x_t": x_t,
                "wpk_t": _prep_wpk(
                    wmu[rs], wrho[rs], weps[rs], mu_sc, rho_sc, rho_min
                ),
                "bpk": np.ascontiguousarray(
                    np.concatenate([bmu[:, rs], brho[:, rs], beps[:, rs]], axis=1)
                ),
            }
        )
    return in_maps


def kernel(x, weight_mu, weight_rho, bias_mu, bias_rho, weight_eps, bias_eps):
    o_sh = OUT_F // N_CORES
    qparams = quant_params(weight_mu, weight_rho)
    key = (x.shape, o_sh, qparams)
    if key not in _NC_CACHE:
        _NC_CACHE[key] = build_nc(x.shape[0], x.shape[1], o_sh, qparams)
    nc = _NC_CACHE[key]

    in_maps = make_in_maps(
        x, weight_mu, weight_rho, bias_mu, bias_rho, weight_eps, bias_eps
    )
    res = run_bass_kernel_spmd(nc, in_maps, core_ids=list(range(N_CORES)))
    return np.concatenate([res.results[c]["y"] for c in range(N_CORES)], axis=1)


# revision 16
# speedup vs baseline: 1.0059x; 1.0059x over previous
"""BayesianLinear (y = x @ (mu + softplus(rho) * eps).T + bias) on 8 TRN2 cores.

Column-parallel sharding: each core owns OUT_F/8 = 512 output features.

Host-side prep is pure layout/precision staging (no reference math):
  - x is cast to bf16 and pre-tiled into the SBUF layout the TensorEngine
    needs for its stationary operand: x_t[bt, pi, po, bi] = x[bt*128+bi,
    po*128+pi], so each 128-row batch tile is one contiguous 1 MiB DMA.
  - weight params are transposed to [in_f, o_sh], tiled per 128-row
    K-block, and PACKED into one uint8 tensor [128, KB, 2048]:
    eps bf16 (1024 B) | mu int8 (512 B) | rho uint8 (512 B) per block.
    mu/rho ship as affine-quantized 8-bit codes (scales in a tiny qp
    tensor): uniform 8-bit beats fp8 ~3x in rms error for Gaussian data
    and halves the packed-weight HBM traffic (12.6 -> 8.4 MiB/core) --
    the construction phase is HBM-bound, not compute-bound.

Device per core:
  1. Construction units (2 single K-blocks first for a short critical
     path, then 15 pairs): one packed DMA (GPSIMD SWDGE queue), then
     softplus(rho) = Ln(1 + Exp(rho)) on ACT -- the uint8 rho dequant
     rides Exp's free affine (out = f(scale*in + bias), scale/bias as
     per-partition APs from qp). DVE: mul by eps (bf16 2x mode), then
     one fused scalar_tensor_tensor (mu_i8 * mu_scale) + sp_eps writing
     bf16 straight into the resident W^T tile [128, 32, 512].
  2. bias row = bias_mu + softplus(bias_rho) * bias_eps (fp32, tiny),
     broadcast across partitions with one K=1 matmul against a ones row
     mid-wave so the in-order PE stream never head-of-line blocks on it.
  3. First 7 batch tiles run as a k-WAVEFRONT across the 7 PSUM banks:
     wave w issues tile i's matmul for k = w - i. Tile 0's first matmul
     only needs W^T block 0 + one 256 KiB x chunk, so real work starts
     ~12 us in (vs ~27 us for the k-major group), and the PE consumes
     W^T blocks no faster than construction produces them. x chunks are
     DMA'd in need-order with a 10-wave lookahead so the packed-weight
     DMAs get their required HBM share.
  4. A short PE warmup (dummy K=1 matmuls, no DMA deps) bridges the
     framework preamble to first-data-ready so the HAM clock gate is
     already at 8/8 when the real stream starts.
  5. Remaining 56 tiles stream one PSUM bank each: one 1 MiB x DMA, 32
     accumulating bf16 matmuls into PSUM [128, 512] fp32, DVE eviction
     fused with the bias add, DMA out.
"""

import numpy as np
import ml_dtypes

import concourse.bacc as bacc
import concourse.mybir as mybir
import concourse.tile as tile
from concourse.bass_utils import run_bass_kernel_spmd

BATCH = 8192
IN_F = 4096
OUT_F = 4096
N_CORES = 8
P = 128

_NC_CACHE = {}

PKB = 2048  # packed bytes per partition per K-block: eps 1024 | mu 512 | rho 512
WARM = 14  # PE warmup matmuls
LOOKAHEAD = 5  # waves of x-chunk DMA lookahead
CH = 4  # x chunks per strip (8 K-blocks each)


def build_nc(batch=BATCH, in_f=IN_F, o_sh=OUT_F // N_CORES, qparams=(1.0, 0.0, 1.0)):
    """qparams = (rho_scale, rho_min, mu_scale), shared by all cores
    (global quantization grid), baked as compile-time immediates so the
    first Exp doesn't wait on a parameter DMA."""
    rho_sc, rho_min, mu_sc = (float(v) for v in qparams)
    KB = in_f // P  # K-blocks of 128 along the contraction dim
    BT = batch // P  # 128-row output tiles

    nc = bacc.Bacc(
        "TRN2",
        target_bir_lowering=False,
        debug=False,
        enable_asserts=False,
        num_devices=N_CORES,
    )
    bf16 = mybir.dt.bfloat16
    f32 = mybir.dt.float32
    u8 = mybir.dt.uint8
    i8 = mybir.dt.int8

    x = nc.declare_dram_parameter("x_t", [BT, P, KB, P], bf16, isOutput=False)
    wpk = nc.declare_dram_parameter("wpk_t", [P, KB, PKB], u8, isOutput=False)
    bpk = nc.declare_dram_parameter("bpk", [1, 3 * o_sh], f32, isOutput=False)
    y = nc.declare_dram_parameter("y", [batch, o_sh], f32, isOutput=True)

    act_exp = mybir.ActivationFunctionType.Exp
    act_ln = mybir.ActivationFunctionType.Ln
    op_mult = mybir.AluOpType.mult
    op_add = mybir.AluOpType.add

    # construction units: small first (short first-ready chain, and the
    # early wavefront demands one fresh block per ~1.1 us), quads later —
    # bigger ACT passes amortize the ~352-cycle fixed cost; the serial
    # ACT chain is the construction floor.
    units = [(0, 1), (1, 1), (2, 2), (4, 2), (6, 2)]
    b = 8
    while b < KB:
        s = min(4, KB - b)
        units.append((b, s))
        b += s

    with tile.TileContext(nc) as tc:
        with (
            tc.tile_pool(name="const", bufs=1) as const,
            tc.tile_pool(name="wcons", bufs=6) as wcons,
            # bufs=8: 7 live group strips + ONE streaming-prefetch slot.
            # More slots would let 1 MiB streaming prefetches flood the
            # DMA rings mid-group and starve the packed-weight DMAs the
            # ACT pipeline is waiting on (measured: wpk latency 11 us,
            # ACT blocked 27 us). Slots open progressively as group
            # strips retire, so streaming runway builds during the
            # group tail exactly when HBM frees up.
            tc.tile_pool(name="xin", bufs=8) as xin,
            tc.tile_pool(name="yout", bufs=4) as yout,
            tc.tile_pool(name="psum", bufs=7, space="PSUM") as psum_pool,
            tc.tile_pool(name="bpsum", bufs=1, space="PSUM") as bias_psum,
        ):
            bias_sb = const.tile([P, o_sh], f32, tag="bias_sb")
            bias_bf = const.tile([1, o_sh], bf16, tag="bias_bf")
            ones = const.tile([1, P], bf16, tag="ones")
            nc.vector.memset(ones[:], 1.0)
            wones = const.tile([1, o_sh], bf16, tag="wones")
            nc.vector.memset(wones[:], 1.0)
            # rho_min as a memset const (activation bias must be an AP;
            # a memset has no DMA dependency, unlike a parameter load)
            rho_min_sb = const.tile([P, 1], f32, tag="rho_min_sb")
            nc.vector.memset(rho_min_sb[:], rho_min)
            # dummy Exp so walrus's ACT_TABLE_LOAD runs during the idle
            # preamble instead of serializing behind the first wpk DMA
            act_scratch = const.tile([P, 1], f32, tag="act_scratch")
            nc.scalar.activation(act_scratch[:], rho_min_sb[:], act_exp)

            # PE warmup: dummy matmuls with no DMA deps bridge the
            # ~6.5us framework preamble to first-data-ready (~12us).
            # K must be 128: the HAM clock gate watches PE-array
            # activity, and a K=1 matmul (1 of 128 rows) stays below its
            # busy threshold — measured traces showed K=1 warmups never
            # flipped the gate and the first ~20 real matmuls ran at the
            # cold 1.2 GHz clock.
            warm_w = const.tile([P, P], bf16, tag="warm_w")
            nc.vector.memset(warm_w[:], 0.001)
            warm_r = const.tile([P, o_sh], bf16, tag="warm_r")
            nc.vector.memset(warm_r[:], 0.001)
            warm_ps = bias_psum.tile([P, o_sh], f32, tag="bias_ps", name="warm_ps")
            for w in range(WARM):
                nc.tensor.matmul(warm_ps[:], lhsT=warm_w[:], rhs=warm_r[:])

            def emit_bias_row():
                # one packed DMA (mu | rho | eps) on the gpsimd ring so
                # the sync ring's head stays dedicated to x chunks
                b_pk = const.tile([1, 3 * o_sh], f32, tag="b_pk")
                nc.gpsimd.dma_start(out=b_pk[:], in_=bpk[:])
                b_mu = b_pk[:, 0:o_sh]
                b_rho = b_pk[:, o_sh : 2 * o_sh]
                b_eps = b_pk[:, 2 * o_sh : 3 * o_sh]
                b_sp = const.tile([1, o_sh], f32, tag="b_sp")
                nc.scalar.activation(b_sp[:], b_rho[:], act_exp)
                nc.scalar.activation(b_sp[:], b_sp[:], act_ln, bias=1.0)
                nc.vector.tensor_mul(out=b_sp[:], in0=b_sp[:], in1=b_eps[:])
                nc.vector.tensor_add(out=bias_bf[:], in0=b_sp[:], in1=b_mu[:])

            # ---- W^T constructed in place. Per unit: one packed DMA,
            # Exp with fused uint8-rho dequant, Ln, eps-mul (bf16 2x),
            # fused (mu_i8 * scale) + sp_eps -> bf16 W^T block.
            WT = const.tile([P, KB, o_sh], bf16, tag="WT")
            for ui, (ub, us) in enumerate(units):
                pk = wcons.tile([P, us, PKB], u8, tag="pk")
                # unit 0 rides the sync ring's head (HWDGE, in front of
                # the x chunks) — it gates the whole construction chain
                dma_eng = nc.sync if ui == 0 else nc.gpsimd
                dma_eng.dma_start(out=pk[:], in_=wpk[:, ub : ub + us, :])
                eps_v = pk[:, :, 0 : 2 * o_sh].bitcast(bf16)
                mu_v = pk[:, :, 2 * o_sh : 3 * o_sh].bitcast(i8)
                rho_v = pk[:, :, 3 * o_sh : 4 * o_sh]
                sp = wcons.tile([P, us, o_sh], bf16, tag="sp")
                nc.scalar.activation(
                    sp[:], rho_v[:], act_exp, bias=rho_min_sb[:], scale=rho_sc
                )
                nc.scalar.activation(sp[:], sp[:], act_ln, bias=1.0)
                nc.vector.tensor_mul(out=sp[:], in0=sp[:], in1=eps_v[:])
                nc.vector.scalar_tensor_tensor(
                    out=WT[:, ub : ub + us, :],
                    in0=mu_v[:],
                    scalar=mu_sc,
                    in1=sp[:],
                    op0=op_mult,
                    op1=op_add,
                )
                if ui == 2:
                    emit_bias_row()

            def body_tail(ps, bt):
                y_sb = yout.tile([P, o_sh], f32, tag="y_sb")
                nc.vector.tensor_add(out=y_sb[:], in0=ps[:], in1=bias_sb[:])
                nc.sync.dma_start(out=y[bt * P : (bt + 1) * P, :], in_=y_sb[:])

            # ---- first GROUP tiles run as a k-wavefront across PSUM
            # banks: wave w = tile i's matmul for k = w - i. Tile 0's
            # k=0 matmul needs only W^T block 0 + one x chunk.
            GROUP = min(7, BT)
            KC = KB // CH  # K-blocks per x chunk
            xts = []
            pss = []
            for bt in range(GROUP):
                xT = xin.tile([P, KB, P], bf16, tag="xT", name=f"xT_g{bt}")
                xts.append(xT)
                ps = psum_pool.tile([P, o_sh], f32, tag="ps", name=f"ps_g{bt}")
                pss.append(ps)

            # x chunk (i, c) is first read at wave i + c*KC; DMA in need
            # order with LOOKAHEAD waves of headroom so the packed-weight
            # DMAs keep their HBM share.
            chunks = sorted(
                ((i + c * KC, i, c) for i in range(GROUP) for c in range(CH))
            )

            def issue_chunks_through(wave):
                while chunks and chunks[0][0] <= wave:
                    _, i, c = chunks.pop(0)
                    ks = slice(c * KC, (c + 1) * KC)
                    nc.sync.dma_start(out=xts[i][:, ks, :], in_=x[i, :, ks, :])

            issue_chunks_through(LOOKAHEAD - 1)
            # ramp fillers: during waves 0-5 the construction pipeline
            # produces one W^T block per ~1.2 us but the young wavefront
            # only has ~0.2-1.3 us of matmuls per wave. Pad the in-order
            # PE queue with no-dep warmup matmuls sized to the expected
            # production lag — the PE never idles, so the HAM clock gate
            # never re-throttles into the dense phase (an idle gap of
            # ~2 us mid-ramp measurably dropped the PE to 1.2 GHz for
            # 17 us).
            FILLERS = {1: 5, 2: 4, 3: 3, 4: 2, 5: 1}
            NWAVE = KB + GROUP - 1
            for w in range(NWAVE):
                issue_chunks_through(w + LOOKAHEAD)
                for _ in range(FILLERS.get(w, 0)):
                    nc.tensor.matmul(warm_ps[:], lhsT=warm_w[:], rhs=warm_r[:])
                for i in range(GROUP):
                    k = w - i
                    if 0 <= k < KB:
                        nc.tensor.matmul(
                            pss[i][:],
                            lhsT=xts[i][:, k, :],
                            rhs=WT[:, k, :],
                            start=(k == 0),
                            stop=(k == KB - 1),
                        )
                if w == 12:
                    # bias broadcast: [128, o_sh] = ones.T @ bias_bf.
                    # Mid-stream so the in-order PE queue never blocks
                    # on the bias chain; ready long before 1st eviction.
                    bias_ps = bias_psum.tile(
                        [P, o_sh], f32, tag="bias_ps", name="bias_ps"
                    )
                    nc.tensor.matmul(bias_ps[:], lhsT=ones[:], rhs=bias_bf[:])
                    nc.vector.tensor_copy(out=bias_sb[:], in_=bias_ps[:])
                gi = w - (KB - 1)
                if 0 <= gi < GROUP:
                    body_tail(pss[gi], gi)

            # ---- remaining tiles stream one PSUM bank each
            for bt in range(GROUP, BT):
                xT = xin.tile([P, KB, P], bf16, tag="xT")
                nc.sync.dma_start(out=xT[:], in_=x[bt])
                ps = psum_pool.tile([P, o_sh], f32, tag="ps")
                for k in range(KB):
                    nc.tensor.matmul(
                        ps[:],
                        lhsT=xT[:, k, :],
                        rhs=WT[:, k, :],
                        start=(k == 0),
                        stop=(k == KB - 1),
                    )
                body_tail(ps, bt)

    # Skip bacc's pre-placed InstLoadActFuncSet: on large graphs walrus's
    # parallel-pass fork can separate the hoisted load from its activations
    # ("No Act func set exist for this instruction"); walrus's own lower_act
    # placement handles forked subgraphs correctly.
    nc.insert_act_table_loads = lambda: None
    nc.compile()
    return nc


def _prep_x(x):
    """[batch, in_f] fp32 -> bf16 tiled [BT, 128, KB, 128] with
    x_t[bt, pi, po, bi] = x[bt*128 + bi, po*128 + pi]."""
    batch, in_f = x.shape
    xb = x.astype(ml_dtypes.bfloat16)
    xb = xb.reshape(batch // P, P, in_f // P, P)  # [bt, bi, po, pi]
    return np.ascontiguousarray(xb.transpose(0, 3, 2, 1))  # [bt, pi, po, bi]


def _tile_w(w):
    """[o_sh, in_f] -> tiled [KB, 128, o_sh] with w_t[k, pi, o] = w[o, k*128 + pi]."""
    o_sh, in_f = w.shape
    return np.ascontiguousarray(w.T.reshape(in_f // P, P, o_sh))


def _prep_wpk(wmu, wrho, weps, mu_sc, rho_sc, rho_min):
    """Pack eps (bf16 bytes), mu (int8 codes), rho (uint8 codes) into one
    uint8 [128, KB, 2048] tensor (global quantization grids)."""
    eps_t = _tile_w(weps).astype(ml_dtypes.bfloat16)  # [KB, P, o]
    mu_t = _tile_w(wmu)
    rho_t = _tile_w(wrho)

    mu_c = np.clip(np.round(mu_t / mu_sc), -127, 127).astype(np.int8)
    rho_c = np.clip(np.round((rho_t - rho_min) / rho_sc), 0, 255).astype(np.uint8)

    kb, p, o = mu_t.shape
    pk = np.concatenate(
        [
            eps_t.view(np.uint8).reshape(kb, p, 2 * o),
            mu_c.view(np.uint8),
            rho_c,
        ],
        axis=2,
    )  # [KB, P, 4*o]
    return np.ascontiguousarray(pk.transpose(1, 0, 2))


def quant_params(weight_mu, weight_rho):
    """Global (all-core) quantization grids for mu/rho — shared so they
    can be baked into the SPMD program as immediates."""
    wmu = np.asarray(weight_mu, dtype=np.float32)
    wrho = np.asarray(weight_rho, dtype=np.float32)
    mu_sc = max(float(np.abs(wmu).max()) / 127.0, 1e-30)
    rmin = float(wrho.min())
    rho_sc = max((float(wrho.max()) - rmin) / 255.0, 1e-30)
    return rho_sc, rmin, mu_sc


def make_in_maps(x, weight_mu, weight_rho, bias_mu, bias_rho, weight_eps, bias_eps):
    o_sh = OUT_F // N_CORES
    rho_sc, rho_min, mu_sc = quant_params(weight_mu, weight_rho)
    x_t = _prep_x(np.asarray(x, dtype=np.float32))
    wmu = np.asarray(weight_mu, dtype=np.float32)
    wrho = np.asarray(weight_rho, dtype=np.float32)
    weps = np.asarray(weight_eps, dtype=np.float32)
    bmu = np.asarray(bias_mu, dtype=np.float32).reshape(1, -1)
    brho = np.asarray(bias_rho, dtype=np.float32).reshape(1, -1)
    beps = np.asarray(bias_eps, dtype=np.float32).reshape(1, -1)

    in_maps = []
    for c in range(N_CORES):
        rs = slice(c * o_sh, (c + 1) * o_sh)
        in_maps.append(
            {
                "x_t": x_t,
                "wpk_t": _prep_wpk(
                    wmu[rs], wrho[rs], weps[rs], mu_sc, rho_sc, rho_min
                ),
                "bpk": np.ascontiguousarray(
                    np.concatenate([bmu[:, rs], brho[:, rs], beps[:, rs]], axis=1)
                ),
            }
        )
    return in_maps


def kernel(x, weight_mu, weight_rho, bias_mu, bias_rho, weight_eps, bias_eps):
    o_sh = OUT_F // N_CORES
    qparams = quant_params(weight_mu, weight_rho)
    key = (x.shape, o_sh, qparams)
    if key not in _NC_CACHE:
        _NC_CACHE[key] = build_nc(x.shape[0], x.shape[1], o_sh, qparams)
    nc = _NC_CACHE[key]

    in_maps = make_in_maps(
        x, weight_mu, weight_rho, bias_mu, bias_rho, weight_eps, bias_eps
    )
    res = run_bass_kernel_spmd(nc, in_maps, core_ids=list(range(N_CORES)))
    return np.concatenate([res.results[c]["y"] for c in range(N_CORES)], axis=1)
